# revision 15
# baseline (speedup 1.0000x reference)
"""Trainium2 Bass kernel for nn_CDEM_62079457296798 (channel-attention
transformer block).

Sharding: 8 cores = 4 batches x 2 spatial halves (64 rows + 1 halo row each).
Cross-core communication: one small AllReduce per core-pair carrying the
channel-attention Gram matrices and q/k l2-norm sums; everything else local.

Layout: channel-major activations [C_part, pixels_free]. Per-head channel
padding 48 -> 64 (256 padded channels) keeps head boundaries 32/64 aligned;
attention path runs in bf16.

Engine split: the q depthwise 3x3 runs on the tensor engine (9 accumulated
diag-block matmuls per 32-channel group, 4-way tile_position packing); the
k and v depthwise 3x3 run on the vector engine as 9 shifted per-partition
multiply-accumulates (bf16 2x mode, with a DMA-shifted twin image so odd
column taps stay 4B-aligned). q/k l2-norm sums come from qq/kk autogram
matmuls on the transposed chunks (diag extracted on DVE). attn is folded
into the linear layer (Wt = Wlin @ attn per head) so z is never formed.
"""
import sys
sys.path.insert(0, '/opt/trn_rl_repo')

import numpy as np
import ml_dtypes

from concourse import bacc, mybir, tile
from concourse.bass_utils import run_bass_kernel_spmd

F32 = mybir.dt.float32
F32R = mybir.dt.float32r
BF16 = mybir.dt.bfloat16
AF = mybir.ActivationFunctionType
OP = mybir.AluOpType
bf16 = ml_dtypes.bfloat16

N_CORES = 8
B, C, H, W = 4, 192, 128, 128
HEADS, CH = 4, 48
CPH = 64                # padded channels per head
CP = HEADS * CPH        # 256 padded attn channels
HLOC = 64               # image rows per core
ER, EC = 66, 130        # ext rows/cols (halo + zero pad)
NEXT = ER * EC          # 8580
NLOC = HLOC * W         # 8192
NCK = 16                # output chunks (4 rows x 128 = 512 px)
CONV_CHUNKS = [(i * 512, 512) for i in range(16)] + [(16 * 512, NEXT - 16 * 512)]
KB = [(0, 128), (128, 64)]          # 192-channel K bands

DIRECT_PSUM_OUT = False


def build_nc():
    nc = bacc.Bacc("TRN2", target_bir_lowering=False, debug=False,
                   num_devices=N_CORES)

    d_xe = nc.dram_tensor("xe", [C, NEXT], BF16, kind="ExternalInput")
    d_ye = nc.dram_tensor("ye", [C, NEXT], BF16, kind="ExternalInput")
    d_wq = nc.dram_tensor("wq", [C, CP], BF16, kind="ExternalInput")
    d_wkv = nc.dram_tensor("wkv", [C, 2 * CP], BF16, kind="ExternalInput")
    d_qdw = nc.dram_tensor("qdw", [CP, 9, 32], BF16, kind="ExternalInput")
    d_kvdwp = nc.dram_tensor("kvdwp", [2 * CP, 9], F32, kind="ExternalInput")
    d_wlin = nc.dram_tensor("wlin", [CP, C], BF16, kind="ExternalInput")
    d_wf1 = nc.dram_tensor("wf1", [C, 768], BF16, kind="ExternalInput")
    d_wf2 = nc.dram_tensor("wf2", [768, C], BF16, kind="ExternalInput")
    d_wpr = nc.dram_tensor("wpr", [C, C], BF16, kind="ExternalInput")
    d_tempb = nc.dram_tensor("tempb", [128, 2], F32, kind="ExternalInput")
    d_alpha = nc.dram_tensor("alpha", [128, 1], F32, kind="ExternalInput")
    d_gamma = nc.dram_tensor("gamma", [128, 1], F32, kind="ExternalInput")
    d_id128 = nc.dram_tensor("id128", [128, 128], F32, kind="ExternalInput")
    d_out = nc.dram_tensor("out", [C, NLOC], F32, kind="ExternalOutput")
    cc_in = nc.dram_tensor("cc_in", [112, 228], F32)
    cc_out = nc.dram_tensor("cc_out", [112, 228], F32)

    with tile.TileContext(nc) as tc:
        with (
            tc.tile_pool(name="sbw", bufs=1) as sbw,      # weights/consts
            tc.tile_pool(name="sbpre", bufs=2) as sbpre,  # conv1x1 out (ext img)
            tc.tile_pool(name="sbtw", bufs=1) as sbtw,    # shifted twin image
            tc.tile_pool(name="sbin", bufs=3) as sbin,    # streamed conv inputs
            tc.tile_pool(name="sbqk", bufs=3) as sbqk,    # q chunk tiles
            tc.tile_pool(name="sbT", bufs=1) as sbT,      # qT/kT/v/kacc persistents
            tc.tile_pool(name="sbs", bufs=1) as sbs,      # small attn tiles
            tc.tile_pool(name="sbc", bufs=2) as sbc,      # trunk chunk pipeline
            tc.tile_pool(name="sbg", bufs=6) as sbg,      # gelu chunk tiles
            tc.tile_pool(name="pcv", bufs=3, space="PSUM") as pcv,
            tc.tile_pool(name="pdw", bufs=2, space="PSUM") as pdw,
            tc.tile_pool(name="pacc", bufs=1, space="PSUM") as pacc,
            tc.tile_pool(name="psm", bufs=2, space="PSUM") as psm,
        ):
            # ---------- weights ----------
            wq_t = [sbw.tile([s, CP], BF16, tag=f"wq{i}", name=f"wq{i}")
                    for i, (o, s) in enumerate(KB)]
            wkv_t = [sbw.tile([s, 2 * CP], BF16, tag=f"wkv{i}", name=f"wkv{i}")
                     for i, (o, s) in enumerate(KB)]
            for i, (o, s) in enumerate(KB):
                nc.sync.dma_start(wq_t[i][:], d_wq[o:o + s, :])
            qdw_t = [sbw.tile([128, 9, 32], BF16, tag=f"qdw{m}", name=f"qdw{m}") for m in range(2)]
            for m in range(2):
                nc.sync.dma_start(qdw_t[m][:], d_qdw[128 * m:128 * (m + 1)])
            # per-partition plain dw weights for DVE blocks (k0,k1,v0,v1)
            kvdwp_t = [sbw.tile([128, 9], F32, tag=f"kvdwp{m}", name=f"kvdwp{m}")
                       for m in range(4)]

            def load_kv_weights():
                for i, (o, s) in enumerate(KB):
                    nc.sync.dma_start(wkv_t[i][:], d_wkv[o:o + s, :])
                for m in range(4):
                    nc.sync.dma_start(kvdwp_t[m][:], d_kvdwp[128 * m:128 * (m + 1)])

            wlin_t = [sbw.tile([128, C], BF16, tag=f"wlin{m}", name=f"wlin{m}") for m in range(2)]
            wf1_t = [sbw.tile([s, 768], BF16, tag=f"wf1{i}", name=f"wf1{i}")
                     for i, (o, s) in enumerate(KB)]
            wf2_t = [sbw.tile([128, C], BF16, tag=f"wf2{k}", name=f"wf2{k}") for k in range(6)]
            wpr_t = [sbw.tile([s, C], BF16, tag=f"wpr{i}", name=f"wpr{i}")
                     for i, (o, s) in enumerate(KB)]
            tempb = sbw.tile([128, 2], F32, tag="tempb", name="tempb")
            alphav = sbw.tile([128, 1], F32, tag="alphav", name="alphav")
            gammav = sbw.tile([128, 1], F32, tag="gammav", name="gammav")
            id128 = sbw.tile([128, 128], F32, tag="id128", name="id128")

            def load_trunk_weights():
                for m in range(2):
                    nc.sync.dma_start(wlin_t[m][:], d_wlin[128 * m:128 * (m + 1), :])
                for i, (o, s) in enumerate(KB):
                    nc.sync.dma_start(wf1_t[i][:], d_wf1[o:o + s, :])
                for k in range(6):
                    nc.sync.dma_start(wf2_t[k][:], d_wf2[128 * k:128 * (k + 1), :])
                for i, (o, s) in enumerate(KB):
                    nc.sync.dma_start(wpr_t[i][:], d_wpr[o:o + s, :])
                nc.sync.dma_start(tempb[:], d_tempb.ap())
                nc.sync.dma_start(alphav[:], d_alpha.ap())
                nc.sync.dma_start(gammav[:], d_gamma.ap())
                nc.sync.dma_start(id128[:], d_id128.ap())

            # persistent attn-path results
            qT = [sbT.tile([128, 64, 112], BF16, tag=f"qT{p}", name=f"qT{p}") for p in range(2)]
            kT = [sbT.tile([128, 64, 112], BF16, tag=f"kT{p}", name=f"kT{p}") for p in range(2)]
            vband = [sbT.tile([128, NLOC], BF16, tag=f"v{m}", name=f"v{m}") for m in range(2)]
            # gram psum: [0:224] qk (p0,p1), [224:336] qq (per-p, drained), [336:448] kk
            gacc = pacc.tile([112, 448], F32, tag="gacc", name="gacc")
            # norm sums (memset to 1 so pad rows stay finite)
            sqv = sbs.tile([128, 2], F32, tag="sqv", name="sqv")
            skv = sbs.tile([128, 2], F32, tag="skv", name="skv")
            nc.vector.memset(sqv[:], 1.0)
            nc.vector.memset(skv[:], 1.0)

            # ---------- conv1x1 producer (chunk-paired for weight reuse) ----------
            def conv_pre(src_dram, w_t, m):
                """conv1x1 of 192 -> padded 128-block m over the ext image.
                Returns the pre tile [128, ER, EC] bf16."""
                pre = sbpre.tile([128, ER, EC], BF16, tag="pre", name="pre")
                pref = pre[:].rearrange("p a b -> p (a b)")
                for cb in range(0, len(CONV_CHUNKS), 2):
                    idxs = [i for i in (cb, cb + 1) if i < len(CONV_CHUNKS)]
                    xcs = []
                    for ci in idxs:
                        c0, cn = CONV_CHUNKS[ci]
                        xc = [sbin.tile([s, 512], BF16, tag=f"xin{i}", name=f"xin{i}")
                              for i, (o, s) in enumerate(KB)]
                        for i, (o, s) in enumerate(KB):
                            nc.sync.dma_start(xc[i][:, :cn],
                                              src_dram[o:o + s, c0:c0 + cn])
                        xcs.append(xc)
                    pss = [pcv.tile([128, 512], F32, tag="cv", name="cv") for _ in idxs]
                    for i in range(2):
                        for j, ci in enumerate(idxs):
                            c0, cn = CONV_CHUNKS[ci]
                            nc.tensor.matmul(pss[j][:, :cn],
                                             w_t[i][:, 128 * m:128 * (m + 1)],
                                             xcs[j][i][:, :cn],
                                             start=(i == 0), stop=(i == 1))
                    for j, ci in enumerate(idxs):
                        c0, cn = CONV_CHUNKS[ci]
                        nc.scalar.copy(pref[:, c0:c0 + cn], pss[j][:, :cn])
                return pre

            # ---------- PE depthwise 3x3 (q path) ----------
            def dw_pe(pre, dw_tile, sink):
                """9-tap depthwise via diag-block matmuls; sink(ck, psum_flat)."""
                for ck in range(NCK):
                    r0 = 1 + 4 * ck
                    dp = pdw.tile([128, 4, 128], F32, tag="dw", name="dw")
                    for t in range(9):
                        dr, dc = t // 3 - 1, t % 3 - 1
                        for g in range(4):
                            nc.tensor.matmul(
                                dp[g * 32:(g + 1) * 32, :, :],
                                dw_tile[g * 32:(g + 1) * 32, t, :],
                                pre[g * 32:(g + 1) * 32,
                                    r0 + dr:r0 + 4 + dr, 1 + dc:129 + dc],
                                start=(t == 0), stop=(t == 8),
                                tile_position=(g * 32, g * 32))
                    sink(ck, dp[:].rearrange("p a b -> p (a b)"))

            # ---------- DVE depthwise 3x3 (k/v path) ----------
            RQ = 16  # image rows per DVE quarter

            def dw_dve_quarter(pre, preB, wp, dst, qv):
                """9-tap depthwise as per-partition MACs on the vector engine.
                dst: [128, RQ*128] bf16 quarter."""
                for t in range(9):
                    dr, dc = t // 3 - 1, t % 3 - 1
                    r0 = 1 + qv * RQ + dr
                    offc = 1 + dc
                    if offc % 2 == 0:
                        src = pre[:, r0:r0 + RQ, offc:offc + 128]
                    else:
                        src = preB[:, r0:r0 + RQ, offc - 1:offc - 1 + 128]
                    if t == 0:
                        nc.vector.tensor_scalar(
                            out=dst, in0=src, scalar1=wp[:, t:t + 1],
                            scalar2=None, op0=OP.mult)
                    else:
                        nc.vector.scalar_tensor_tensor(
                            out=dst, in0=src, scalar=wp[:, t:t + 1],
                            in1=dst, op0=OP.mult, op1=OP.add)

            def make_twin(pre):
                """preB[p, i] = pre[p, i+1] via SBUF->SBUF DMA (parity twin)."""
                preB = sbtw.tile([128, ER, EC], BF16, tag="preB", name="preB")
                pf = pre[:].rearrange("p a b -> p (a b)")
                pBf = preB[:].rearrange("p a b -> p (a b)")
                nc.sync.dma_start(pBf[:, 0:NEXT - 1], pf[:, 1:NEXT])
                return preB

            # ---------- q/k sinks: evac + transpose + autogram ----------
            def gram_mms(p, ck, is_q):
                for j in range(4):
                    sub = 4 * ck + j
                    if is_q:
                        nc.tensor.matmul(
                            gacc[:, 224:336], qT[p][:, sub, :], qT[p][:, sub, :],
                            start=(sub == 0), stop=(sub == 63))
                    else:
                        nc.tensor.matmul(
                            gacc[:, 112 * p:112 * (p + 1)],
                            qT[p][:, sub, :], kT[p][:, sub, :],
                            start=(sub == 0), stop=(sub == 63))
                        nc.tensor.matmul(
                            gacc[:, 336:448], kT[p][:, sub, :], kT[p][:, sub, :],
                            start=(sub == 0), stop=(sub == 63))

            def drain_diag(region, dst, p):
                """dst[0:112, p] = diag(gacc[:, region]) via id-mask + reduce."""
                tmp = sbs.tile([112, 112], F32, tag="dgt", name="dgt")
                nc.vector.tensor_tensor(out=tmp[:], in0=gacc[:, region[0]:region[1]],
                                        in1=id128[0:112, 0:112], op=OP.mult)
                nc.vector.tensor_reduce(dst[0:112, p:p + 1], tmp[:],
                                        axis=mybir.AxisListType.X, op=OP.add)

            # ============ q production (PE dwconv) ============
            load_kv_weights()
            load_trunk_weights()
            for m in range(2):
                pre = conv_pre(d_xe, wq_t, m)

                def q_sink(ck, flat, m=m):
                    qc = sbqk.tile([128, 512], BF16, tag="qkc", name="qkc")
                    if ck % 2 == 0:
                        nc.vector.tensor_copy(qc[:], flat)
                    else:
                        nc.scalar.copy(qc[:], flat)
                    nc.sync.dma_start_transpose(
                        qT[m][:, 4 * ck:4 * ck + 4, :], qc[0:112, :])
                    gram_mms(m, ck, is_q=True)
                dw_pe(pre, qdw_t[m], q_sink)
                drain_diag((224, 336), sqv, m)

            # ============ k production (DVE dwconv) ============
            for m in range(2):
                pre = conv_pre(d_ye, wkv_t, m)
                preB = make_twin(pre)
                for qv in range(HLOC // RQ):
                    kq = sbqk.tile([128, RQ * 128], BF16, tag="kq", name="kq")
                    dw_dve_quarter(pre, preB, kvdwp_t[m], kq[:], qv)
                    for j in range(4):
                        ck = 4 * qv + j
                        nc.sync.dma_start_transpose(
                            kT[m][:, 4 * ck:4 * ck + 4, :],
                            kq[0:112, 512 * j:512 * j + 512])
                        gram_mms(m, ck, is_q=False)
                drain_diag((336, 448), skv, m)

            # ============ pair AllReduce ============
            gsb = sbs.tile([112, 224], F32, tag="gsb", name="gsb")
            nc.vector.tensor_copy(gsb[:], gacc[:, 0:224])
            nc.sync.dma_start(cc_in.ap()[:, 0:224], gsb[:])
            nc.sync.dma_start(cc_in.ap()[:, 224:226], sqv[0:112, :])
            nc.sync.dma_start(cc_in.ap()[:, 226:228], skv[0:112, :])
            nc.gpsimd.collective_compute(
                "AllReduce", OP.add,
                replica_groups=[[0, 1], [2, 3], [4, 5], [6, 7]],
                ins=[cc_in.ap()], outs=[cc_out.ap()])

            # ============ v production (DVE dwconv), overlaps AllReduce ============
            for m in range(2):
                pre = conv_pre(d_ye, wkv_t, 2 + m)
                preB = make_twin(pre)
                for qv in range(HLOC // RQ):
                    o0 = qv * RQ * 128
                    dw_dve_quarter(pre, preB, kvdwp_t[2 + m],
                                   vband[m][:, o0:o0 + RQ * 128], qv)

            gg = sbs.tile([112, 224], F32, tag="gg", name="gg")
            sqg = sbs.tile([128, 2], F32, tag="sqg", name="sqg")
            skg = sbs.tile([128, 2], F32, tag="skg", name="skg")
            nc.vector.memset(sqg[:], 1.0)
            nc.vector.memset(skg[:], 1.0)
            nc.sync.dma_start(gg[:], cc_out.ap()[:, 0:224])
            nc.sync.dma_start(sqg[0:112, :], cc_out.ap()[:, 224:226])
            nc.sync.dma_start(skg[0:112, :], cc_out.ap()[:, 226:228])

            # ============ attention finalize ============
            def rsqrt_newton(tag, s_t):
                sc = sbs.tile([128, 2], F32, tag=tag + "_c")
                nc.vector.tensor_scalar_max(sc[:], s_t[:], 1e-24)
                rt = sbs.tile([128, 2], F32, tag=tag + "_s")
                nc.scalar.activation(rt[:], sc[:], AF.Sqrt)
                r0 = sbs.tile([128, 2], F32, tag=tag + "_r0")
                nc.vector.reciprocal(r0[:], rt[:])
                rr = sbs.tile([128, 2], F32, tag=tag + "_rr")
                nc.vector.tensor_tensor(out=rr[:], in0=r0[:], in1=r0[:], op=OP.mult)
                t1_ = sbs.tile([128, 2], F32, tag=tag + "_t1")
                nc.vector.scalar_tensor_tensor(out=t1_[:], in0=sc[:], scalar=-0.5,
                                               in1=rr[:], op0=OP.mult, op1=OP.mult)
                nc.vector.tensor_scalar_add(t1_[:], t1_[:], 1.5)
                rv = sbs.tile([128, 2], F32, tag=tag)
                nc.vector.tensor_tensor(out=rv[:], in0=r0[:], in1=t1_[:], op=OP.mult)
                return rv

            rq = rsqrt_newton("rq", sqg)
            rk = rsqrt_newton("rk", skg)
            srow = sbs.tile([128, 2], F32, tag="srow", name="srow")
            nc.vector.tensor_tensor(out=srow[:], in0=rq[:], in1=tempb[:], op=OP.mult)

            srow_r, scol_r = [], []
            for p in range(2):
                for src, lst, nm in ((srow, srow_r, "sr"), (rk, scol_r, "sc")):
                    fp = psm.tile([1, 112], F32, tag="sm", name="sm")
                    nc.tensor.transpose(fp[:], src[0:112, p:p + 1],
                                        id128[0:112, 0:112])
                    fr = sbs.tile([1, 112], F32R, tag=f"{nm}{p}", name=f"{nm}{p}")
                    nc.vector.tensor_copy(fr[:], fp[:])
                    lst.append(fr)

            # Wt = Wlin @ attn per head slot, stored as lhsT [pad-d, 192] bf16
            wt_sb = [sbs.tile([128, C], BF16, tag=f"wt{p}", name=f"wt{p}") for p in range(2)]
            for p in range(2):
                nc.gpsimd.memset(wt_sb[p][:], 0.0)
            for p in range(2):
                spair = psm.tile([112, 112], F32, tag="sm", name="sm")
                nc.tensor.matmul(spair[:], srow_r[p][:], scol_r[p][:],
                                 start=True, stop=True)
                lg = sbs.tile([112, 112], F32, tag="lg", name="lg")
                nc.vector.tensor_tensor(out=lg[:], in0=gg[:, 112 * p:112 * (p + 1)],
                                        in1=spair[:], op=OP.mult)
                at16 = sbs.tile([112, 112], BF16, tag="at16", name="at16")
                for e in range(2):
                    sl = slice(64 * e, 64 * e + 48)
                    mx = sbs.tile([112, 1], F32, tag="mx", name="mx")
                    nc.vector.tensor_reduce(mx[sl, :], lg[sl, sl],
                                            axis=mybir.AxisListType.X, op=OP.max)
                    exh = sbs.tile([112, 112], F32, tag="exh", name="exh")
                    nc.vector.tensor_scalar(out=exh[sl, 0:48], in0=lg[sl, sl],
                                            scalar1=mx[sl, :], scalar2=None,
                                            op0=OP.subtract)
                    ex2 = sbs.tile([112, 112], F32, tag="ex2", name="ex2")
                    den = sbs.tile([112, 1], F32, tag="den", name="den")
                    nc.scalar.activation(ex2[sl, 0:48], exh[sl, 0:48], AF.Exp,
                                         accum_out=den[sl, :])
                    rc0 = sbs.tile([112, 1], F32, tag="rc0", name="rc0")
                    nc.vector.reciprocal(rc0[sl, :], den[sl, :])
                    nt = sbs.tile([112, 1], F32, tag="nt", name="nt")
                    nc.vector.tensor_tensor(out=nt[sl, :], in0=den[sl, :],
                                            in1=rc0[sl, :], op=OP.mult)
                    nc.vector.tensor_scalar(out=nt[sl, :], in0=nt[sl, :],
                                            scalar1=-1.0, scalar2=2.0,
                                            op0=OP.mult, op1=OP.add)
                    rc1 = sbs.tile([112, 1], F32, tag="rc1", name="rc1")
                    nc.vector.tensor_tensor(out=rc1[sl, :], in0=rc0[sl, :],
                                            in1=nt[sl, :], op=OP.mult)
                    nc.vector.tensor_scalar(out=at16[sl, 0:48], in0=ex2[sl, 0:48],
                                            scalar1=rc1[sl, :], scalar2=None,
                                            op0=OP.mult)
                    # Wt slot: [48 pad-d rows at 64e] x 192, lhsT=attn[c,d] slice
                    wtp = psm.tile([128, C], F32, tag="sm", name="wtp")
                    nc.tensor.matmul(wtp[64 * e:64 * e + 48, :],
                                     at16[sl, 0:48], wlin_t[p][sl, :],
                                     start=True, stop=True)
                    nc.scalar.copy(wt_sb[p][64 * e:64 * e + 48, :],
                                   wtp[64 * e:64 * e + 48, :])

            # ============ per-chunk trunk ============
            for ck in range(NCK):
                c0 = ck * 512
                tp = [pcv.tile([128, 512], F32, tag="cv", name="cv") for _ in range(2)]
                for mi, (mo, ms) in enumerate(KB):
                    for k2 in range(2):
                        nc.tensor.matmul(tp[mi][:ms, :],
                                         wt_sb[k2][:, mo:mo + ms],
                                         vband[k2][:, c0:c0 + 512],
                                         start=(k2 == 0), stop=(k2 == 1))
                ycn = sbc.tile([128, 2, 512], BF16, tag="ycn", name="ycn")
                for mi, (mo, ms) in enumerate(KB):
                    yv = d_ye[mo:mo + ms, :].rearrange(
                        "c (a b) -> c a b", a=ER)[:, 1 + 4 * ck:5 + 4 * ck, 1:129]
                    nc.sync.dma_start(ycn[:ms, mi, :], yv)
                t1c = [sbc.tile([s, 512], BF16, tag=f"t1c{i}", name=f"t1c{i}")
                       for i, (o, s) in enumerate(KB)]
                for mi, (mo, ms) in enumerate(KB):
                    nc.vector.scalar_tensor_tensor(
                        out=t1c[mi][:], in0=ycn[:ms, mi, :], scalar=alphav[:ms, :],
                        in1=tp[mi][:ms, :], op0=OP.mult, op1=OP.add)
                gc = [sbg.tile([128, 512], BF16, tag="gc", name="gc") for _ in range(6)]
                for mt in range(6):
                    fp1 = pcv.tile([128, 512], F32, tag="cv", name="cv")
                    for i in range(2):
                        nc.tensor.matmul(fp1[:], wf1_t[i][:, 128 * mt:128 * (mt + 1)],
                                         t1c[i][:], start=(i == 0), stop=(i == 1))
                    nc.scalar.activation(gc[mt][:], fp1[:], AF.Gelu)
                t2c = [sbc.tile([s, 512], BF16, tag=f"t2c{i}", name=f"t2c{i}")
                       for i, (o, s) in enumerate(KB)]
                for mi, (mo, ms) in enumerate(KB):
                    fp2 = pcv.tile([128, 512], F32, tag="cv", name="cv")
                    for k in range(6):
                        nc.tensor.matmul(fp2[:ms, :], wf2_t[k][:, mo:mo + ms],
                                         gc[k][:], start=(k == 0), stop=(k == 5))
                    nc.vector.scalar_tensor_tensor(
                        out=t2c[mi][:], in0=t1c[mi][:], scalar=gammav[:ms, :],
                        in1=fp2[:ms, :], op0=OP.mult, op1=OP.add)
                for mi, (mo, ms) in enumerate(KB):
                    pp = pcv.tile([128, 512], F32, tag="cv", name="cv")
                    for i in range(2):
                        nc.tensor.matmul(pp[:ms, :], wpr_t[i][:, mo:mo + ms],
                                         t2c[i][:], start=(i == 0), stop=(i == 1))
                    if DIRECT_PSUM_OUT:
                        nc.sync.dma_start(d_out[mo:mo + ms, c0:c0 + 512],
                                          pp[:ms, :])
                    else:
                        oc = sbc.tile([128, 512], F32, tag=f"oc{mi}", name=f"oc{mi}")
                        nc.scalar.copy(oc[:ms, :], pp[:ms, :])
                        nc.sync.dma_start(d_out[mo:mo + ms, c0:c0 + 512],
                                          oc[:ms, :])

    nc.compile()
    return nc


_NC = None


def _get_nc():
    global _NC
    if _NC is None:
        _NC = build_nc()
    return _NC


def _prep_weights(q_w, q_dw_w, kv_w, kv_dw_w, linear_w, proj_w, ffn1_w, ffn2_w,
                  temperature, alpha, beta, gamma, delta):
    def pad_oc(w):  # [192 real oc, ic] -> [ic, 256 padded oc]
        out = np.zeros((C, CP), np.float32)
        for h in range(HEADS):
            out[:, CPH * h:CPH * h + CH] = w[CH * h:CH * (h + 1), :].T
        return out

    wq = pad_oc(np.asarray(q_w, np.float32))
    kv = np.asarray(kv_w, np.float32)
    wkv = np.concatenate([pad_oc(kv[:C]), pad_oc(kv[C:])], axis=1)

    def pad_dw(w):  # [192,1,3,3] -> [256, 9, 32] diag blocks
        out = np.zeros((CP, 9, 32), np.float32)
        for h in range(HEADS):
            for j in range(CH):
                cp = CPH * h + j
                out[cp, :, cp % 32] = w[CH * h + j, 0].reshape(9)
        return out

    def pad_dw_plain(w):  # [192,1,3,3] -> [256, 9] per-channel taps
        out = np.zeros((CP, 9), np.float32)
        for h in range(HEADS):
            for j in range(CH):
                out[CPH * h + j, :] = w[CH * h + j, 0].reshape(9)
        return out

    qdw = pad_dw(np.asarray(q_dw_w, np.float32))
    kvd = np.asarray(kv_dw_w, np.float32)
    kvdwp = np.concatenate([pad_dw_plain(kvd[:C]), pad_dw_plain(kvd[C:])], axis=0)

    lin = np.asarray(linear_w, np.float32) * float(beta)
    wlin = np.zeros((CP, C), np.float32)
    for h in range(HEADS):
        wlin[CPH * h:CPH * h + CH, :] = lin[:, CH * h:CH * (h + 1)].T

    wf1 = np.asarray(ffn1_w, np.float32).T.copy()
    wf2 = (np.asarray(ffn2_w, np.float32) * float(delta)).T.copy()
    wpr = np.asarray(proj_w, np.float32).T.copy()

    tempb = np.zeros((128, 2), np.float32)
    tv = np.asarray(temperature, np.float32).reshape(HEADS)
    for h in range(HEADS):
        tempb[64 * (h % 2):64 * (h % 2) + 64, h // 2] = tv[h]

    alphav = np.full((128, 1), float(alpha), np.float32)
    gammav = np.full((128, 1), float(gamma), np.float32)
    id128 = np.eye(128, dtype=np.float32)

    return {
        "wq": wq.astype(bf16), "wkv": wkv.astype(bf16),
        "qdw": qdw.astype(bf16), "kvdwp": kvdwp,
        "wlin": wlin.astype(bf16), "wf1": wf1.astype(bf16),
        "wf2": wf2.astype(bf16), "wpr": wpr.astype(bf16),
        "tempb": tempb, "alpha": alphav, "gamma": gammav,
        "id128": id128,
    }


def kernel(**inputs):
    x = np.asarray(inputs["x"], np.float32)
    y = np.asarray(inputs["y"], np.float32)
    shared = _prep_weights(
        inputs["q_w"], inputs["q_dw_w"], inputs["kv_w"], inputs["kv_dw_w"],
        inputs["linear_w"], inputs["proj_w"], inputs["ffn1_w"], inputs["ffn2_w"],
        inputs["temperature"], inputs["alpha"], inputs["beta"],
        inputs["gamma"], inputs["delta"])

    in_maps = []
    for c in range(N_CORES):
        bi, s = c // 2, c % 2
        r0 = s * HLOC
        xe = np.zeros((C, ER, EC), np.float32)
        ye = np.zeros((C, ER, EC), np.float32)
        rlo, rhi = max(r0 - 1, 0), min(r0 + HLOC + 1, H)
        elo = rlo - (r0 - 1)
        xe[:, elo:elo + (rhi - rlo), 1:129] = x[bi, :, rlo:rhi, :]
        ye[:, elo:elo + (rhi - rlo), 1:129] = y[bi, :, rlo:rhi, :]
        m = dict(shared)
        m["xe"] = xe.reshape(C, NEXT).astype(bf16)
        m["ye"] = ye.reshape(C, NEXT).astype(bf16)
        in_maps.append(m)

    nc = _get_nc()
    res = run_bass_kernel_spmd(nc, in_maps, list(range(N_CORES)))
    out = np.empty((B, C, H, W), np.float32)
    for c in range(N_CORES):
        bi, s = c // 2, c % 2
        out[bi, :, s * HLOC:(s + 1) * HLOC, :] = \
            res.results[c]["out"].reshape(C, HLOC, W)
    return out
